# revision 1
# baseline (speedup 1.0000x reference)
"""GNN message-passing decoder kernel for 8 trn2 cores.

Strategy: shard the N (residue) dimension across the 8 cores; each core
processes 512 (b, n) rows. Geometry/topk/features computed per shard, MLP
weights replicated. The heavy edge-MLP matmuls run as a Bass SPMD kernel on
the 8 NeuronCores (fp32r matmuls, verified primitives); the remaining glue
(kNN selection, geometric features, layernorms) runs on host in fp32.
"""
import numpy as np

H = 128
K = 30
NUM_RBF = 16
POS = 16
SEQN = 30
DEPTH = 3
B = 2
N = 2048
NC = 8


def _norm(x):
    ssq = np.clip((x * x).sum(-1, keepdims=True, dtype=np.float32), 1e-24, None)
    return (x / np.sqrt(ssq)).astype(np.float32)


def _safe_sqrt(x):
    p = x > 0
    return np.where(p, np.sqrt(np.where(p, x, 1.0)), 0.0).astype(np.float32)


def _gather(nodes, idx):
    # nodes [B,N,C], idx [B,N,K] -> [B,N,K,C]
    return np.stack([nodes[b][idx[b]] for b in range(nodes.shape[0])], 0)


def _ln(x, g, b, eps=1e-6):
    mu = x.mean(-1, keepdims=True, dtype=np.float32)
    var = ((x - mu) ** 2).sum(-1, keepdims=True, dtype=np.float32) / (x.shape[-1] - 1)
    return (g * (x - mu) / (np.sqrt(var + eps) + eps) + b).astype(np.float32)


def _edge_mlp_device(h, h_e, E_idx, vmask, m, Wl1, bl1, Wl2, bl2, Wl3, bl3, gl, bl):
    """3 MPNN layers. Runs the per-edge MLP matmuls on the 8 NeuronCores via
    a Bass SPMD kernel when available; falls back to host numpy otherwise."""
    try:
        return _edge_mlp_bass(h, h_e, E_idx, vmask, m, Wl1, bl1, Wl2, bl2, Wl3, bl3, gl, bl)
    except Exception:
        return _edge_mlp_host(h, h_e, E_idx, vmask, m, Wl1, bl1, Wl2, bl2, Wl3, bl3, gl, bl)


def _edge_mlp_host(h, h_e, E_idx, vmask, m, Wl1, bl1, Wl2, bl2, Wl3, bl3, gl, bl):
    for l in range(DEPTH):
        nei_v = _gather(h, E_idx)
        h_EV = np.concatenate(
            [np.broadcast_to(h[:, :, None, :], nei_v.shape), nei_v, h_e], -1)
        msg = np.maximum(h_EV @ Wl1[l] + bl1[l], 0.0)
        msg = np.maximum(msg @ Wl2[l] + bl2[l], 0.0)
        msg = (msg @ Wl3[l] + bl3[l]) * vmask[..., None]
        h = _ln(h + msg.mean(-2, dtype=np.float32), gl[l], bl[l]) * m[:, :, None]
        h = h.astype(np.float32)
    return h


_BASS_CACHE = {}


def _edge_mlp_bass(h, h_e, E_idx, vmask, m, Wl1, bl1, Wl2, bl2, Wl3, bl3, gl, bl):
    """Device path: each core owns 512 (b,n) rows (N/8 per batch). Per layer,
    host does the (cheap) neighbor gather into transposed activations; the
    three 384/128/128-deep matmuls + relus for 15360 edges per core run on
    device; host finishes mean-over-K + LN (small: [4096, 128])."""
    import concourse.bass as bass
    import concourse.mybir as mybir
    import concourse.tile as tile
    import concourse.bacc as bacc
    from concourse.bass_utils import run_bass_kernel_spmd

    F32 = mybir.dt.float32
    R = mybir.dt.float32r
    AF = mybir.ActivationFunctionType
    ROWS = B * N // NC          # 512 rows per core
    EDG = ROWS * K              # 15360 edges per core
    NB = EDG // 512             # 30 blocks of 512 edge-columns

    if "nc" not in _BASS_CACHE:
        nc = bacc.Bacc(num_devices=NC)
        x_in = nc.dram_tensor("x", [384, EDG], F32, kind="ExternalInput")
        w_in = nc.dram_tensor("w", [384 + H + H, H], F32, kind="ExternalInput")
        o_out = nc.dram_tensor("o", [H, EDG], F32, kind="ExternalOutput")
        with tile.TileContext(nc) as tc:
            with (
                tc.tile_pool(name="p", bufs=2) as pool,
                tc.tile_pool(name="wp", bufs=1) as wpool,
                tc.tile_pool(name="ps", bufs=2, space="PSUM") as psum,
            ):
                wr = wpool.tile([384 + H + H, H], R)
                wf = wpool.tile([384 + H + H, H], F32)
                nc.sync.dma_start(wf[:], w_in[:])
                nc.vector.tensor_copy(wr[:], wf[:])
                for bk in range(NB):
                    xb = pool.tile([384, 512], F32, tag="xb")
                    nc.sync.dma_start(xb[:], x_in[:, 512 * bk:512 * (bk + 1)])
                    xr = pool.tile([384, 512], R, tag="xr")
                    nc.vector.tensor_copy(xr[:], xb[:])
                    p1 = psum.tile([128, 512], F32, tag="p1")
                    for c in range(3):
                        nc.tensor.matmul(
                            p1[:], wr[128 * c:128 * (c + 1), :],
                            xr[128 * c:128 * (c + 1), :],
                            start=(c == 0), stop=(c == 2))
                    m1 = pool.tile([128, 512], R, tag="m1")
                    nc.scalar.activation(m1[:], p1[:], AF.Relu)
                    p2 = psum.tile([128, 512], F32, tag="p2")
                    nc.tensor.matmul(p2[:], wr[384:384 + H, :], m1[:])
                    m2 = pool.tile([128, 512], R, tag="m2")
                    nc.scalar.activation(m2[:], p2[:], AF.Relu)
                    p3 = psum.tile([128, 512], F32, tag="p3")
                    nc.tensor.matmul(p3[:], wr[384 + H:, :], m2[:])
                    m3 = pool.tile([128, 512], F32, tag="m3")
                    nc.vector.tensor_copy(m3[:], p3[:])
                    nc.sync.dma_start(o_out[:, 512 * bk:512 * (bk + 1)], m3[:])
        nc.compile()
        _BASS_CACHE["nc"] = nc
    nc = _BASS_CACHE["nc"]

    for l in range(DEPTH):
        nei_v = _gather(h, E_idx)                      # [B,N,K,H]
        hc = np.broadcast_to(h[:, :, None, :], nei_v.shape)
        # per-core transposed activations [384, EDG]
        in_maps = []
        w_all = np.concatenate([Wl1[l], Wl2[l], Wl3[l]], 0).astype(np.float32)
        for c in range(NC):
            sl = slice(c * N // NC, (c + 1) * N // NC)
            xc = np.concatenate(
                [hc[:, sl], nei_v[:, sl], h_e[:, sl]], -1)   # [B,256,K,384]
            in_maps.append({
                "x": np.ascontiguousarray(
                    xc.reshape(EDG, 384).T.astype(np.float32)),
                "w": w_all,
            })
        res = run_bass_kernel_spmd(nc, in_maps, list(range(NC)))
        msg = np.concatenate(
            [res.results[c]["o"].T.reshape(B, N // NC, K, H) for c in range(NC)],
            axis=1)                                     # [B,N,K,H]
        msg = (msg + bl3[l]) * vmask[..., None]
        h = _ln(h + msg.mean(-2, dtype=np.float32), gl[l], bl[l]) * m[:, :, None]
        h = h.astype(np.float32)
    return h


def kernel(X, mask, Wv_w, Wv_b, gv, bv, We_w, We_b, ge, be,
           Wl1, bl1, Wl2, bl2, Wl3, bl3, gl, bl):
    X = np.asarray(X, np.float32)
    mask = np.asarray(mask, np.float32)
    Wv_w = np.asarray(Wv_w, np.float32); Wv_b = np.asarray(Wv_b, np.float32)
    gv = np.asarray(gv, np.float32); bv = np.asarray(bv, np.float32)
    We_w = np.asarray(We_w, np.float32); We_b = np.asarray(We_b, np.float32)
    ge = np.asarray(ge, np.float32); be = np.asarray(be, np.float32)
    Wl1 = np.asarray(Wl1, np.float32); bl1 = np.asarray(bl1, np.float32)
    Wl2 = np.asarray(Wl2, np.float32); bl2 = np.asarray(bl2, np.float32)
    Wl3 = np.asarray(Wl3, np.float32); bl3 = np.asarray(bl3, np.float32)
    gl = np.asarray(gl, np.float32); bl = np.asarray(bl, np.float32)

    Bv, Nv = X.shape[0], X.shape[2]
    m = mask.reshape(Bv, -1)
    Xc = X[:, :, :, 1, :].reshape(Bv, -1, 3)
    # ---- kNN graph ----
    m2 = np.clip(m[:, :, None] * m[:, None, :] - np.eye(Nv, dtype=np.float32), 0.0, None)
    dP = Xc[:, :, None, :] - Xc[:, None, :, :]
    D = m2 * np.sqrt((dP * dP).sum(-1, dtype=np.float32) + 1e-6)
    Dmask = (D + (1.0 - m2) * 10000.0).astype(np.float32)
    E_idx = np.argsort(Dmask, axis=-1, kind="stable")[:, :, :K]
    D_nb = np.take_along_axis(Dmask, E_idx, axis=-1)
    # ---- RBF ----
    mu_r = np.linspace(0.0, 20.0, NUM_RBF, dtype=np.float32)
    RBF = np.exp(-(((D_nb[..., None] - mu_r) / (20.0 / NUM_RBF)) ** 2)).astype(np.float32)
    # ---- positional encoding ----
    ii = np.arange(Nv, dtype=np.float32)[None, :, None]
    d = (E_idx.astype(np.float32) - ii)[..., None] * m[:, :, None, None]
    d = np.where(np.abs(d) > SEQN, 0.0, d).astype(np.float32)
    freq = np.exp(np.arange(0, POS, 2, dtype=np.float32) * (-np.log(10000.0) / POS))
    ang = d * freq
    Ep = (np.concatenate([np.cos(ang), np.sin(ang)], -1) * (d != 0)).astype(np.float32)
    # ---- orientation features ----
    U = _norm((Xc[:, 1:] - Xc[:, :-1]) * m[:, 1:, None])
    u2, u1 = U[:, :-2], U[:, 1:-1]
    n2 = _norm(np.cross(u2, u1))
    o1 = _norm(u2 - u1)
    O = np.stack([o1, n2, np.cross(o1, n2)], 2).reshape(Bv, Nv - 3, 9)
    O = np.pad(O, ((0, 0), (1, 2), (0, 0))).astype(np.float32)
    mN = m[:, :, None, None]
    O_nb = _gather(O, E_idx) * mN
    X_nb = _gather(Xc, E_idx) * mN
    Om = O.reshape(Bv, Nv, 3, 3)
    Onb = O_nb.reshape(Bv, Nv, K, 3, 3)
    dXn = (X_nb - Xc[:, :, None, :]) * mN
    dU = _norm(np.einsum("bnij,bnkj->bnki", Om, dXn).astype(np.float32))
    Rm = np.einsum("bnji,bnkjl->bnkil", Om, Onb).astype(np.float32)
    Rxx, Ryy, Rzz = Rm[..., 0, 0], Rm[..., 1, 1], Rm[..., 2, 2]
    mags = 0.5 * _safe_sqrt(np.abs(1.0 + np.stack(
        [Rxx - Ryy - Rzz, -Rxx + Ryy - Rzz, -Rxx - Ryy + Rzz], -1)))
    signs = np.sign(np.stack(
        [Rm[..., 2, 1] - Rm[..., 1, 2], Rm[..., 0, 2] - Rm[..., 2, 0],
         Rm[..., 1, 0] - Rm[..., 0, 1]], -1)).astype(np.float32)
    w = _safe_sqrt(np.maximum(1.0 + Rxx + Ryy + Rzz, 0.0))[..., None] / 2.0
    Q = _norm(np.concatenate([signs * mags, w], -1))
    Of = (np.concatenate([dU, Q], -1) * mN).astype(np.float32)
    # ---- dihedral features ----
    Xd = X.reshape(Bv, Nv, 4, 3)[:, :, :3, :].reshape(Bv, 3 * Nv, 3)
    me = np.repeat(m[:, :, None], 3, axis=2).reshape(Bv, -1)
    Ud = _norm((Xd[:, 1:] - Xd[:, :-1]) * me[:, 1:, None])
    u_2, u_1, u_0 = Ud[:, :-2], Ud[:, 1:-1], Ud[:, 2:]
    n_2 = _norm(np.cross(u_2, u_1)); n_1 = _norm(np.cross(u_1, u_0))
    cosD = np.clip((n_2 * n_1).sum(-1, dtype=np.float32), -1.0 + 1e-7, 1.0 - 1e-7)
    Dang = np.sign((u_2 * n_1).sum(-1, dtype=np.float32)) * np.arccos(cosD)
    Dang = np.pad(Dang, ((0, 0), (1, 2))).reshape(Bv, Nv, 3)
    V = (np.concatenate([np.cos(Dang), np.sin(Dang)], -1) * m[:, :, None]).astype(np.float32)
    E = np.concatenate([Ep, RBF, Of], -1).astype(np.float32)
    # ---- encoder ----
    vmask = _gather(m[:, :, None], E_idx)[..., 0]
    h = _ln(V @ Wv_w + Wv_b, gv, bv)
    h_e = _ln(E @ We_w + We_b, ge, be)
    h = _edge_mlp_device(h, h_e, E_idx, vmask, m,
                         Wl1, bl1, Wl2, bl2, Wl3, bl3, gl, bl)
    return h.astype(np.float32)



# revision 18
# speedup vs baseline: 1.3594x; 1.3594x over previous
"""GNN message-passing decoder: full single-dispatch Bass kernel for 8 trn2 cores.

Sharding: 4096 (b,n) rows split 512/core (core c: batch c//4, rows (c%4)*512).
On device per core: pairwise -d^2 via 5-contraction matmul, bf16 top-32 with
DVE max/max_index/match_replace (slot 0 = self, dropped), int16 edge-index
tile, dma_gather of a 64-float geometry table (coords+orientation+seq idx),
edge features (RBF/positional/orientation-quaternion) in a (row%4,k)x(row//4)
layout, PE-transposed into E^T, edge-LN folded into the layer-1 MLP weights,
3 MPNN layers with transposed bf16 dma_gather of h, mean-over-K folded into
W3, node LN via ones-matmul stats, in-kernel AllGather of h between layers.
Host only builds tables/weight folds (numpy) and reshapes the output.
"""
import numpy as np

H = 128
K = 30
KP = 32
NUM_RBF = 16
POS = 16
SEQN = 30
DEPTH = 3
B = 2
N = 2048
NCORES = 8
NPC = 512            # rows per core
EDGE = NPC * KP      # 16384 padded edges per core
TWO_PI = float(2.0 * np.pi)

_CACHE = {}


# ---------------------------------------------------------------- host numpy
def _norm(x):
    ssq = np.clip((x * x).sum(-1, keepdims=True, dtype=np.float32), 1e-24, None)
    return (x / np.sqrt(ssq)).astype(np.float32)


def _host_geometry(Xc, m):
    """O orientation table [B,N,9] (reference lines 45-50)."""
    Bv, Nv = Xc.shape[0], Xc.shape[1]
    U = _norm((Xc[:, 1:] - Xc[:, :-1]) * m[:, 1:, None])
    u2, u1 = U[:, :-2], U[:, 1:-1]
    n2 = _norm(np.cross(u2, u1))
    o1 = _norm(u2 - u1)
    O = np.stack([o1, n2, np.cross(o1, n2)], 2).reshape(Bv, Nv - 3, 9)
    return np.pad(O, ((0, 0), (1, 2), (0, 0))).astype(np.float32)


def _host_dihedral(X, m):
    """V node features [B,N,6] (reference lines 66-74)."""
    Bv = X.shape[0]
    Nv = X.shape[2]
    Xd = X.reshape(Bv, Nv, 4, 3)[:, :, :3, :].reshape(Bv, 3 * Nv, 3)
    me = np.repeat(m[:, :, None], 3, axis=2).reshape(Bv, -1)
    Ud = _norm((Xd[:, 1:] - Xd[:, :-1]) * me[:, 1:, None])
    u_2, u_1, u_0 = Ud[:, :-2], Ud[:, 1:-1], Ud[:, 2:]
    n_2 = _norm(np.cross(u_2, u_1))
    n_1 = _norm(np.cross(u_1, u_0))
    cosD = np.clip((n_2 * n_1).sum(-1, dtype=np.float32), -1.0 + 1e-7, 1.0 - 1e-7)
    Dang = np.sign((u_2 * n_1).sum(-1, dtype=np.float32)) * np.arccos(cosD)
    Dang = np.pad(Dang, ((0, 0), (1, 2))).reshape(Bv, Nv, 3)
    V = (np.concatenate([np.cos(Dang), np.sin(Dang)], -1) * m[:, :, None])
    return V.astype(np.float32)


# ------------------------------------------------------------- bass program
def _build_program():
    import concourse.mybir as mybir
    import concourse.tile as tile
    import concourse.bacc as bacc
    from concourse.masks import make_identity

    F32 = mybir.dt.float32
    R = mybir.dt.float32r
    BF = mybir.dt.bfloat16
    I16 = mybir.dt.int16
    U16 = mybir.dt.uint16
    I32 = mybir.dt.int32
    AF = mybir.ActivationFunctionType
    ALU = mybir.AluOpType
    AX = mybir.AxisListType

    nc = bacc.Bacc(num_devices=NCORES)

    din = {}
    def dt_in(name, shape, dt):
        din[name] = nc.dram_tensor(name, shape, dt, kind="ExternalInput")
        return din[name]

    dlhs = dt_in("dlhs", [5, NPC], R)
    drhs = dt_in("drhs", [5, N], R)
    geot = dt_in("geot", [B * N, 64], F32)
    xo4 = dt_in("xo4", [4, 13 * 128], R)
    sel4 = dt_in("sel4", [4, 128], R)
    vt = dt_in("vt", [7, NPC], R)
    wv = dt_in("wv", [7, H], R)
    we = dt_in("we", [40, H], BF)
    w1a = dt_in("w1a", [H, 3 * H], R)
    w1b = dt_in("w1b", [H, 3 * H], BF)
    w1c = dt_in("w1c", [H, 3 * H], BF)
    w2 = dt_in("w2", [H, 3 * H], R)
    w3 = dt_in("w3", [H, 3 * H], BF)
    bias = dt_in("bias", [H, 17], F32)
    rowofs = dt_in("rowofs", [H, 1], F32)
    hout = nc.dram_tensor("hout", [NPC, H], F32, kind="ExternalOutput")

    RG = [list(range(NCORES))]

    with tile.TileContext(nc) as tc:
        with (
            nc.allow_low_precision(reason="bf16/f32r within 2e-2 tolerance"),
            tc.tile_pool(name="wp", bufs=1) as wp,
            tc.tile_pool(name="sp", bufs=2) as sp,
            tc.tile_pool(name="zp", bufs=4) as zp,
            tc.tile_pool(name="hp", bufs=2) as hp,
            tc.tile_pool(name="pa", bufs=2, space="PSUM") as pa,
            tc.tile_pool(name="pb", bufs=1, space="PSUM") as pb,
            tc.tile_pool(name="dr", bufs=1, space="DRAM") as dr,
        ):
            # ---------------- S0: constants + weights
            ident = wp.tile([128, 128], R, tag="ident")
            make_identity(nc, ident[:])
            identb = wp.tile([128, 128], BF, tag="identb")
            nc.vector.tensor_copy(identb[:], ident[:])
            ones_col = wp.tile([128, 1], R, tag="ones_col")
            nc.vector.memset(ones_col[:], 1.0)
            onesr = wp.tile([1, 128], R, tag="onesr")
            nc.vector.memset(onesr[:], 1.0)
            negr = wp.tile([1, 128], R, tag="negr")
            nc.vector.memset(negr[:], -1.0)
            selbc = wp.tile([20, 8 * 128], R, tag="selbc")
            nc.vector.memset(selbc[:], 0.0)
            for cc in range(4):
                nc.vector.memset(
                    selbc[8 + cc:9 + cc, 128 * cc:128 * (cc + 1)], 1.0)
                nc.vector.memset(
                    selbc[12 + cc:13 + cc, 128 * (4 + cc):128 * (5 + cc)], 1.0)
            selln = wp.tile([7, 2 * 128], R, tag="selln")
            nc.vector.memset(selln[:], 0.0)
            nc.vector.memset(selln[5:6, 0:128], 1.0)
            nc.vector.memset(selln[6:7, 128:256], 1.0)
            cst = wp.tile([128, 17], F32, tag="cst")
            nc.vector.memset(cst[:, 0:1], 1e-6)
            mu_r = np.linspace(0.0, 20.0, NUM_RBF)
            for f in range(NUM_RBF):
                nc.vector.memset(cst[:, 1 + f:2 + f], float(-0.8 * mu_r[f]))

            def load_r(name, shape, tag):
                t1 = wp.tile(shape, R, tag=tag)
                nc.sync.dma_start(t1[:], din[name][:])
                return t1

            def load_bf(name, shape, tag):
                t1 = wp.tile(shape, BF, tag=tag)
                nc.sync.dma_start(t1[:], din[name][:])
                return t1

            dlhs_sb = load_r("dlhs", [5, NPC], "dlhs")
            drhs_sb = load_r("drhs", [5, N], "drhs")
            xo4_sb = load_r("xo4", [4, 13 * 128], "xo4")
            sel4_sb = load_r("sel4", [4, 128], "sel4")
            vt_sb = load_r("vt", [7, NPC], "vt")
            wv_sb = load_r("wv", [7, H], "wv")
            w1a_sb = load_r("w1a", [H, 3 * H], "w1a")
            w2_sb = load_r("w2", [H, 3 * H], "w2")
            we_sb = load_bf("we", [40, H], "we")
            w1b_sb = load_bf("w1b", [H, 3 * H], "w1b")
            w1c_sb = load_bf("w1c", [H, 3 * H], "w1c")
            w3_sb = load_bf("w3", [H, 3 * H], "w3")
            bias_sb = wp.tile([H, 17], F32, tag="bias")
            nc.sync.dma_start(bias_sb[:], bias[:])
            rowofs_sb = wp.tile([H, 1], F32, tag="rowofs")
            nc.sync.dma_start(rowofs_sb[:], rowofs[:])

            h_slab = dr.tile([NPC, H], BF, tag="hs")
            h_table = dr.tile([B * N, H], BF, tag="ht")

            # ---------------- shared: transposed-layout LayerNorm
            def ln_t(y_sb, gamma, beta, out_tile):
                """out = LN over partition dim of y_sb [128,512] (R dtype)."""
                y2 = sp.tile([128, NPC], R, tag="lny2")
                nc.scalar.activation(y2[:], y_sb[:], AF.Square)
                st_ps = pb.tile([33, NPC], F32, tag="stat")
                nc.tensor.matmul(st_ps[0:1, :], ones_col[:], y_sb[:])
                nc.tensor.matmul(st_ps[32:33, :], ones_col[:], y2[:])
                sm = sp.tile([7, NPC], R, tag="lnsm")
                st = sm[0:2, :]
                nc.gpsimd.tensor_copy(st[0:1, :], st_ps[0:1, :])
                nc.gpsimd.tensor_copy(st[1:2, :], st_ps[32:33, :])
                muex = sm[2:4, :]
                nc.vector.tensor_scalar(muex, st, 1.0 / H, None,
                                        op0=ALU.mult)
                msq = sm[4:5, :]
                nc.gpsimd.tensor_tensor(msq, sm[2:3, :], sm[2:3, :],
                                        op=ALU.mult)
                nc.vector.tensor_tensor(msq, sm[3:4, :], msq,
                                        op=ALU.subtract)
                nc.scalar.activation(msq, msq, AF.Sqrt,
                                     bias=cst[0:1, 0:1], scale=float(H / (H - 1.0)))
                nc.vector.tensor_scalar(msq, msq, 1e-6, None, op0=ALU.add)
                inv = sm[5:6, :]
                nc.vector.reciprocal(inv, msq)
                minv = sm[6:7, :]
                nc.gpsimd.tensor_tensor(minv, sm[2:3, :], inv, op=ALU.mult)
                bc_m = pa.tile([128, NPC], F32, tag="bc")
                nc.tensor.matmul(bc_m[:], selln[:, 128:256], sm)
                bc_i = pa.tile([128, NPC], F32, tag="bc")
                nc.tensor.matmul(bc_i[:], selln[:, 0:128], sm)
                t1 = sp.tile([128, NPC], F32, tag="lnt1")
                nc.vector.tensor_tensor(t1[:], y_sb[:], bc_i[:], op=ALU.mult)
                nc.gpsimd.tensor_tensor(t1[:], t1[:], bc_m[:], op=ALU.subtract)
                nc.scalar.activation(out_tile[:], t1[:], AF.Identity,
                                     bias=beta, scale=gamma)

            def emit_h_out(h_sb, last):
                tp_ps = pa.tile([128, NPC], R, tag="tp")
                for g in range(4):
                    nc.tensor.transpose(tp_ps[:, 128 * g:128 * (g + 1)],
                                        h_sb[:, 128 * g:128 * (g + 1)], ident[:])
                if last:
                    hrm = sp.tile([128, NPC], F32, tag="hrmf")
                    nc.gpsimd.tensor_copy(hrm[:], tp_ps[:])
                    dst = hout[:].rearrange("(g p) h -> p g h", p=128)
                    nc.sync.dma_start(dst, hrm[:].rearrange(
                        "p (g h) -> p g h", g=4))
                else:
                    hrm = sp.tile([128, NPC], BF, tag="hrmb")
                    nc.gpsimd.tensor_copy(hrm[:], tp_ps[:])
                    dst = h_slab[:].rearrange("(g p) h -> p g h", p=128)
                    nc.sync.dma_start(dst, hrm[:].rearrange(
                        "p (g h) -> p g h", g=4))
                    nc.gpsimd.collective_compute(
                        "AllGather", ALU.bypass, replica_groups=RG,
                        ins=[h_slab[:].opt()], outs=[h_table[:].opt()])

            # ---------------- S1: h0 = LN(V @ Wv + b)
            z_ps = pa.tile([128, NPC], F32, tag="mm")
            nc.tensor.matmul(z_ps[:], wv_sb[:], vt_sb[:])
            y0 = sp.tile([128, NPC], R, tag="y")
            nc.scalar.activation(y0[:], z_ps[:], AF.Copy)
            h_sb = hp.tile([128, NPC], R, tag="h")
            ln_t(y0, bias_sb[:, 15:16], bias_sb[:, 16:17], h_sb)
            emit_h_out(h_sb, last=False)

            # ---------------- S2: distances + topk + gather indices
            idx16 = wp.tile([128, EDGE // 16], I16, tag="idx16")
            for g in range(4):
                S_g = wp.tile([128, N], BF, tag="Sg", bufs=2, name=f"S{g}")
                for j in range(4):
                    d_ps = pa.tile([128, 512], F32, tag="mm")
                    nc.tensor.matmul(
                        d_ps[:], dlhs_sb[:, 128 * g:128 * (g + 1)],
                        drhs_sb[:, 512 * j:512 * (j + 1)])
                    nc.scalar.activation(S_g[:, 512 * j:512 * (j + 1)],
                                         d_ps[:], AF.Copy)
                maxv = sp.tile([128, 8], BF, tag="maxv")
                idxu = sp.tile([128, KP], U16, tag="idxu")
                for r in range(4):
                    nc.vector.max(out=maxv[:], in_=S_g[:])
                    nc.vector.max_index(out=idxu[:, 8 * r:8 * r + 8],
                                        in_max=maxv[:], in_values=S_g[:])
                    if r < 3:
                        nc.vector.match_replace(
                            out=S_g[:], in_to_replace=maxv[:],
                            in_values=S_g[:], imm_value=-1e6)
                idxf = sp.tile([128, KP], R, tag="idxf")
                nc.vector.tensor_scalar(idxf[:], idxu[:], rowofs_sb[:, 0:1],
                                        None, op0=ALU.add)
                tp_ps = pa.tile([KP, 128], R, tag="tp")
                nc.tensor.transpose(tp_ps[:], idxf[:], ident[:])
                ev = idx16[0:16, 256 * g:256 * (g + 1)].rearrange(
                    "p (c two) -> p two c", two=2)
                nc.vector.tensor_copy(ev[:, 0, :], tp_ps[1:17, :])
                nc.vector.tensor_copy(ev[0:14, 1, :], tp_ps[17:31, :])
                nc.vector.memset(ev[14:16, 1, :], 0)
            nc.vector.tensor_copy(idx16[16:32, :], idx16[0:16, :])
            nc.vector.tensor_copy(idx16[32:64, :], idx16[0:32, :])
            nc.vector.tensor_copy(idx16[64:128, :], idx16[0:64, :])

            # ---------------- S3: geometry gather
            geo = wp.tile([128, 128, 64], F32, tag="gnei")
            nc.gpsimd.dma_gather(
                out_ap=geo[:], in_ap=geot[:], idxs_ap=idx16[:],
                num_idxs=EDGE, num_idxs_reg=EDGE, elem_size=64)

            # ---------------- S4: i-side broadcast tiles [128,(13),128]
            xi = wp.tile([128, 13, 128], F32, tag="xi")
            for grp in range(4):
                nco = min(4, 13 - 4 * grp)
                bc_ps = pa.tile([128, 512], F32, tag="bc")
                for cc in range(nco):
                    comp = 4 * grp + cc
                    nc.tensor.matmul(
                        bc_ps[:, 128 * cc:128 * (cc + 1)], sel4_sb[:],
                        xo4_sb[:, 128 * comp:128 * (comp + 1)])
                nc.gpsimd.tensor_copy(
                    xi[:, 4 * grp:4 * grp + nco, :], bc_ps[:, :128 * nco])

            # ---------------- S5: edge features -> Escr [128, c, 40]
            W1 = wp.tile([128, 16, 128], F32, tag="W1")
            W2 = wp.tile([128, 9, 128], F32, tag="W2")
            Wi = wp.tile([128, 1, 128], I32, tag="Wi")
            Es = wp.tile([128, 128, 40], BF, tag="Es")

            def gslice(comp):
                return geo[:, :, comp:comp + 1].rearrange("p c one -> p (c one)")

            def eslot(f):
                return Es[:, :, f:f + 1].rearrange("p c one -> p (c one)")

            # dX, Ssq, D
            for c3 in range(3):
                nc.gpsimd.tensor_tensor(W1[:, c3, :], gslice(c3),
                                        xi[:, c3, :], op=ALU.subtract)
            nc.gpsimd.tensor_tensor(W2[:, 0:3, :], W1[:, 0:3, :], W1[:, 0:3, :],
                                    op=ALU.mult)
            nc.vector.tensor_reduce(W1[:, 3, :],
                                    W2[:, 0:3, :].transpose([0, 2, 1]),
                                    axis=AX.X, op=ALU.add)
            nc.scalar.activation(W1[:, 4, :], W1[:, 3, :], AF.Sqrt,
                                 bias=cst[:, 0:1])
            # RBF
            for f in range(NUM_RBF):
                t = W1[:, 5 + (f % 2), :]
                nc.scalar.activation(t, W1[:, 4, :], AF.Square,
                                     bias=cst[:, 1 + f:2 + f], scale=0.8)
                nc.scalar.activation(eslot(16 + f), t, AF.Exp, scale=-1.0)
            # positional: d = j - i, clip |d|>30 -> 0, mask d!=0
            nc.gpsimd.tensor_tensor(W1[:, 7, :], gslice(12), xi[:, 12, :],
                                    op=ALU.subtract)
            nc.scalar.activation(W1[:, 8, :], W1[:, 7, :], AF.Abs)
            nc.vector.tensor_scalar(W1[:, 9, :], W1[:, 8, :], float(SEQN),
                                    None, op0=ALU.is_le)
            nc.gpsimd.tensor_tensor(W1[:, 5, :], W1[:, 7, :], W1[:, 9, :],
                                    op=ALU.mult)          # dm
            nc.gpsimd.tensor_tensor(W1[:, 6, :], W1[:, 8, :], W1[:, 9, :],
                                    op=ALU.mult)          # |dm|
            nc.vector.tensor_scalar(W1[:, 8, :], W1[:, 6, :], 0.0, None,
                                    op0=ALU.is_gt)        # mnz
            freq = np.exp(np.arange(0, POS, 2, dtype=np.float64)
                          * (-np.log(10000.0) / POS))
            for q in range(POS // 2):
                fq = float(freq[q])
                # sin(dm*fq)
                nc.vector.tensor_scalar(W2[:, 0, :], W1[:, 5, :],
                                        fq / TWO_PI, None, op0=ALU.mult)
                nc.vector.tensor_copy(Wi[:, 0, :], W2[:, 0, :])
                nc.vector.tensor_copy(W2[:, 1, :], Wi[:, 0, :])
                nc.gpsimd.tensor_tensor(W2[:, 0, :], W2[:, 0, :], W2[:, 1, :],
                                        op=ALU.subtract)
                nc.scalar.activation(eslot(8 + q), W2[:, 0, :], AF.Sin,
                                     scale=TWO_PI)
                # cos(dm*fq) * mnz
                nc.vector.tensor_scalar(W2[:, 2, :], W1[:, 5, :], fq / TWO_PI,
                                        0.25, op0=ALU.mult, op1=ALU.add)
                nc.vector.tensor_copy(Wi[:, 0, :], W2[:, 2, :])
                nc.vector.tensor_copy(W2[:, 1, :], Wi[:, 0, :])
                nc.gpsimd.tensor_tensor(W2[:, 2, :], W2[:, 2, :], W2[:, 1, :],
                                        op=ALU.subtract)
                nc.scalar.activation(W2[:, 1, :], W2[:, 2, :], AF.Sin,
                                     scale=TWO_PI)
                nc.gpsimd.tensor_tensor(eslot(q), W2[:, 1, :], W1[:, 8, :],
                                        op=ALU.mult)
            # dU = norm(Om_i . dX)
            for v in range(3):
                nc.gpsimd.tensor_tensor(W2[:, 0:3, :], xi[:, 3 + 3 * v:6 + 3 * v, :],
                                        W1[:, 0:3, :], op=ALU.mult)
                nc.vector.tensor_reduce(W1[:, 9 + v, :],
                                        W2[:, 0:3, :].transpose([0, 2, 1]),
                                        axis=AX.X, op=ALU.add)
            nc.gpsimd.tensor_tensor(W2[:, 0:3, :], W1[:, 9:12, :],
                                    W1[:, 9:12, :], op=ALU.mult)
            nc.vector.tensor_reduce(W2[:, 3, :],
                                    W2[:, 0:3, :].transpose([0, 2, 1]),
                                    axis=AX.X, op=ALU.add)
            nc.vector.tensor_scalar(W2[:, 3, :], W2[:, 3, :], 1e-24, None,
                                    op0=ALU.max)
            nc.scalar.activation(W2[:, 4, :], W2[:, 3, :], AF.Sqrt)
            nc.vector.reciprocal(W2[:, 5, :], W2[:, 4, :])
            for v in range(3):
                nc.gpsimd.tensor_tensor(eslot(32 + v), W1[:, 9 + v, :],
                                        W2[:, 5, :], op=ALU.mult)
            # R = Om_i^T . O_j  (R[i,l] = sum_j Om[j,i] Onb[j,l])
            om9 = xi[:, 3:12, :].rearrange("p (j i) c -> p j i c", i=3)
            for i in range(3):
                for j in range(3):
                    omji = om9[:, j, i:i + 1, :].rearrange("p one c -> p (one c)")
                    onb = geo[:, :, 3 + 3 * j:6 + 3 * j].transpose([0, 2, 1])
                    if j == 0:
                        nc.gpsimd.tensor_tensor(
                            W2[:, 3 * i:3 * i + 3, :],
                            omji.unsqueeze(1).to_broadcast([128, 3, 128]),
                            onb, op=ALU.mult)
                    else:
                        nc.gpsimd.tensor_tensor(
                            W1[:, 0:3, :],
                            omji.unsqueeze(1).to_broadcast([128, 3, 128]),
                            onb, op=ALU.mult)
                        nc.gpsimd.tensor_tensor(
                            W2[:, 3 * i:3 * i + 3, :], W2[:, 3 * i:3 * i + 3, :],
                            W1[:, 0:3, :], op=ALU.add)
            # quaternion
            Rxx, Ryy, Rzz = W2[:, 0, :], W2[:, 4, :], W2[:, 8, :]
            diag = [(Rxx, Ryy, Rzz, W2[:, 7, :], W2[:, 5, :]),
                    (Ryy, Rxx, Rzz, W2[:, 2, :], W2[:, 6, :]),
                    (Rzz, Rxx, Ryy, W2[:, 3, :], W2[:, 1, :])]
            for a, (pp, m1, m2, sA, sB) in enumerate(diag):
                nc.gpsimd.tensor_tensor(W1[:, 0, :], pp, m1, op=ALU.subtract)
                nc.gpsimd.tensor_tensor(W1[:, 0, :], W1[:, 0, :], m2,
                                        op=ALU.subtract)
                nc.scalar.activation(W1[:, 1, :], W1[:, 0, :], AF.Abs, bias=1.0)
                nc.scalar.activation(W1[:, 0, :], W1[:, 1, :], AF.Sqrt)
                nc.gpsimd.tensor_tensor(W1[:, 1, :], sA, sB, op=ALU.subtract)
                nc.scalar.activation(W1[:, 2, :], W1[:, 1, :], AF.Sign)
                nc.gpsimd.tensor_tensor(W1[:, 12 + a, :], W1[:, 0, :],
                                        W1[:, 2, :], op=ALU.mult)
            nc.gpsimd.tensor_tensor(W1[:, 0, :], Rxx, Ryy, op=ALU.add)
            nc.gpsimd.tensor_tensor(W1[:, 0, :], W1[:, 0, :], Rzz, op=ALU.add)
            nc.scalar.activation(W1[:, 1, :], W1[:, 0, :], AF.Relu, bias=1.0)
            nc.scalar.activation(W1[:, 15, :], W1[:, 1, :], AF.Sqrt)
            nc.gpsimd.tensor_tensor(W2[:, 0:4, :], W1[:, 12:16, :],
                                    W1[:, 12:16, :], op=ALU.mult)
            nc.vector.tensor_reduce(W2[:, 4, :],
                                    W2[:, 0:4, :].transpose([0, 2, 1]),
                                    axis=AX.X, op=ALU.add)
            nc.vector.tensor_scalar(W2[:, 4, :], W2[:, 4, :], 1e-24, None,
                                    op0=ALU.max)
            nc.scalar.activation(W2[:, 5, :], W2[:, 4, :], AF.Sqrt)
            nc.vector.reciprocal(W2[:, 6, :], W2[:, 5, :])
            for a in range(4):
                nc.gpsimd.tensor_tensor(eslot(35 + a), W1[:, 12 + a, :],
                                        W2[:, 6, :], op=ALU.mult)
            nc.vector.memset(eslot(39), 1.0)

            # ---------------- S6+S7: E^T transposes interleaved with
            # u_e = edge-LN(E @ We + be), no affine (folded into W1c/b1)
            ue = wp.tile([128, EDGE], BF, tag="ue")
            for sg in range(8):
                ET = wp.tile([40, 4 * 512], BF, tag="ET", bufs=2,
                             name=f"ET{sg}")
                for grp in range(4):
                    tp_ps = pa.tile([40, 512], BF, tag="tp")
                    for cc in range(4):
                        c = 16 * sg + 4 * grp + cc
                        nc.tensor.transpose(tp_ps[:, 128 * cc:128 * (cc + 1)],
                                            Es[:, c, :], identb[:])
                    nc.gpsimd.tensor_copy(ET[:, 512 * grp:512 * (grp + 1)],
                                          tp_ps[:])
                zs = []
                esm = sp.tile([20, 512], R, tag="esm")
                stats = esm[0:8, :]
                for cc in range(4):
                    z_ps = pa.tile([128, 512], F32, tag="mm")
                    nc.tensor.matmul(z_ps[:], we_sb[:],
                                     ET[:, 512 * cc:512 * (cc + 1)])
                    z_sb = zp.tile([128, 512], R, tag="z")
                    nc.scalar.activation(z_sb[:], z_ps[:], AF.Copy)
                    z2 = sp.tile([128, 512], R, tag="z2")
                    nc.gpsimd.tensor_tensor(z2[:], z_sb[:], z_sb[:],
                                            op=ALU.mult)
                    st_ps = pb.tile([33, 512], F32, tag="stat")
                    nc.tensor.matmul(st_ps[0:1, :], ones_col[:], z_sb[:])
                    nc.tensor.matmul(st_ps[32:33, :], ones_col[:], z2[:])
                    nc.gpsimd.tensor_copy(stats[cc:cc + 1, :], st_ps[0:1, :])
                    nc.gpsimd.tensor_copy(stats[4 + cc:5 + cc, :],
                                          st_ps[32:33, :])
                    zs.append(z_sb)
                muex = esm[8:16, :]
                nc.vector.tensor_scalar(muex, stats, 1.0 / H, None,
                                        op0=ALU.mult)
                msq = esm[16:20, :]
                nc.gpsimd.tensor_tensor(msq, esm[8:12, :], esm[8:12, :],
                                        op=ALU.mult)
                nc.vector.tensor_tensor(msq, esm[12:16, :], msq,
                                        op=ALU.subtract)
                nc.scalar.activation(msq, msq, AF.Sqrt,
                                     bias=cst[0:4, 0:1],
                                     scale=float(H / (H - 1.0)))
                nc.vector.tensor_scalar(msq, msq, 1e-6, None,
                                        op0=ALU.add)
                inv4 = esm[12:16, :]
                nc.vector.reciprocal(inv4, msq)
                nc.gpsimd.tensor_tensor(esm[8:12, :], esm[8:12, :],
                                        inv4, op=ALU.mult)
                for cc in range(4):
                    cg = 4 * sg + cc
                    bc_m = pa.tile([128, 512], F32, tag="bc")
                    nc.tensor.matmul(bc_m[:], selbc[:, 128 * cc:128 * (cc + 1)],
                                     esm[:])
                    bc_i = pa.tile([128, 512], F32, tag="bc")
                    nc.tensor.matmul(
                        bc_i[:], selbc[:, 128 * (4 + cc):128 * (5 + cc)],
                        esm[:])
                    t1 = sp.tile([128, 512], F32, tag="et1")
                    nc.vector.tensor_tensor(t1[:], zs[cc][:], bc_i[:],
                                            op=ALU.mult)
                    nc.gpsimd.tensor_tensor(
                        ue[:, 512 * cg:512 * (cg + 1)], t1[:], bc_m[:],
                        op=ALU.subtract)

            # ---------------- S8: MPNN layers
            for l in range(DEPTH):
                nei = wp.tile([128, 1, EDGE], BF, tag="gnei")
                nc.gpsimd.dma_gather(
                    out_ap=nei[:], in_ap=h_table[:], idxs_ap=idx16[:],
                    num_idxs=EDGE, num_idxs_reg=EDGE, elem_size=H,
                    transpose=True)
                m2s = wp.tile([128, 512], BF, tag="m2s")
                b1 = bias_sb[:, l:l + 1]
                b2 = bias_sb[:, 3 + l:4 + l]
                b3 = bias_sb[:, 6 + l:7 + l]
                gl = bias_sb[:, 9 + l:10 + l]
                bl = bias_sb[:, 12 + l:13 + l]
                wsl = slice(128 * l, 128 * (l + 1))
                for c in range(32):
                    hi_rhs = h_sb[:, 16 * c:16 * (c + 1)].unsqueeze(
                        2).to_broadcast([128, 16, KP])
                    M1 = pa.tile([128, 512], F32, tag="mm")
                    nc.tensor.matmul(M1[:], w1a_sb[:, wsl], hi_rhs,
                                     start=True, stop=False)
                    nc.tensor.matmul(M1[:], w1b_sb[:, wsl],
                                     nei[:, 0, 512 * c:512 * (c + 1)],
                                     start=False, stop=False)
                    nc.tensor.matmul(M1[:], w1c_sb[:, wsl],
                                     ue[:, 512 * c:512 * (c + 1)],
                                     start=False, stop=True)
                    m1 = sp.tile([128, 512], R, tag="m1")
                    nc.scalar.activation(m1[:], M1[:], AF.Relu, bias=b1)
                    M2 = pa.tile([128, 512], F32, tag="mm")
                    nc.tensor.matmul(M2[:], w2_sb[:, wsl], m1[:])
                    m2 = sp.tile([128, 512], BF, tag="m2")
                    nc.scalar.activation(m2[:], M2[:], AF.Relu, bias=b2)
                    nc.vector.tensor_reduce(
                        m2s[:, 16 * c:16 * (c + 1)],
                        m2[:].rearrange("p (r k) -> p r k", k=KP)[:, :, 0:K],
                        axis=AX.X, op=ALU.add)
                msum = pb.tile([128, NPC], F32, tag="msum")
                nc.tensor.matmul(msum[:], w3_sb[:, wsl], m2s[:])
                y_sb = sp.tile([128, NPC], R, tag="y")
                nc.gpsimd.scalar_tensor_tensor(
                    y_sb[:], msum[:], b3, h_sb[:], op0=ALU.add, op1=ALU.add)
                h_sb = hp.tile([128, NPC], R, tag="h")
                ln_t(y_sb, gl, bl, h_sb)
                emit_h_out(h_sb, last=(l == DEPTH - 1))

    nc.compile()
    return nc


# ------------------------------------------------------------------- kernel
def _prep_inputs(X, mask, Wv_w, Wv_b, gv, bv, We_w, We_b, ge, be,
                 Wl1, bl1, Wl2, bl2, Wl3, bl3, gl, bl):
    import ml_dtypes
    bf16 = ml_dtypes.bfloat16
    f32 = np.float32

    m = mask.reshape(B, N).astype(f32)
    Xc = np.ascontiguousarray(X[:, 0, :, 1, :]).astype(f32)    # [B,N,3]
    O = _host_geometry(Xc, m)                                   # [B,N,9]
    V = _host_dihedral(X.astype(f32), m)                        # [B,N,6]
    r2 = (Xc * Xc).sum(-1).astype(f32)                          # [B,N]

    geot = np.zeros((B * N, 64), f32)
    geot[:, 0:3] = Xc.reshape(-1, 3)
    geot[:, 3:12] = O.reshape(-1, 9)
    geot[:, 12] = np.tile(np.arange(N, dtype=f32), B)

    sel4 = np.zeros((4, 128), f32)
    for p in range(128):
        sel4[p // 32, p] = 1.0

    # weight folds
    we = np.zeros((40, H), f32)
    we[0:39] = We_w
    we[39] = We_b
    w1a = np.zeros((H, 3 * H), f32)
    w1b = np.zeros((H, 3 * H), f32)
    w1c = np.zeros((H, 3 * H), f32)
    w2 = np.zeros((H, 3 * H), f32)
    w3 = np.zeros((H, 3 * H), f32)
    bias = np.zeros((H, 17), f32)
    for l in range(DEPTH):
        w1a[:, 128 * l:128 * (l + 1)] = Wl1[l][0:128]
        w1b[:, 128 * l:128 * (l + 1)] = Wl1[l][128:256]
        w1c[:, 128 * l:128 * (l + 1)] = ge[:, None] * Wl1[l][256:384]
        w2[:, 128 * l:128 * (l + 1)] = Wl2[l]
        w3[:, 128 * l:128 * (l + 1)] = Wl3[l] / float(K)
        bias[:, l] = bl1[l] + Wl1[l][256:384].T @ be
        bias[:, 3 + l] = bl2[l]
        bias[:, 6 + l] = bl3[l]
        bias[:, 9 + l] = gl[l]
        bias[:, 12 + l] = bl[l]
    bias[:, 15] = gv
    bias[:, 16] = bv

    shared = {
        "geot": geot, "sel4": sel4,
        "wv": np.concatenate([Wv_w, Wv_b[None]], 0).astype(f32),
        "we": we.astype(bf16),
        "w1a": w1a, "w1b": w1b.astype(bf16), "w1c": w1c.astype(bf16),
        "w2": w2, "w3": w3.astype(bf16), "bias": bias,
    }

    in_maps = []
    for c in range(NCORES):
        b = c // 4
        n0 = (c % 4) * NPC
        rows = slice(n0, n0 + NPC)
        dlhs = np.zeros((5, NPC), f32)
        dlhs[0:3] = 2.0 * Xc[b, rows].T
        dlhs[3] = -r2[b, rows]
        dlhs[4] = -1.0
        drhs = np.zeros((5, N), f32)
        drhs[0:3] = Xc[b].T
        drhs[3] = 1.0
        drhs[4] = r2[b]
        vals = np.zeros((13, NPC), f32)
        vals[0:3] = Xc[b, rows].T
        vals[3:12] = O[b, rows].T
        vals[12] = np.arange(n0, n0 + NPC, dtype=f32)
        xo4 = np.zeros((4, 13 * 128), f32)
        for comp in range(13):
            xo4[:, 128 * comp:128 * (comp + 1)] = vals[comp].reshape(128, 4).T
        vt = np.ones((7, NPC), f32)
        vt[0:6] = V[b, rows].T
        rowofs = np.full((H, 1), b * N, f32)
        mp = {"dlhs": dlhs, "drhs": drhs, "xo4": xo4, "vt": vt,
              "rowofs": rowofs}
        mp.update(shared)
        in_maps.append(mp)
    return in_maps


def _kernel_device(inputs):
    from concourse.bass_utils import run_bass_kernel_spmd
    if "nc" not in _CACHE:
        _CACHE["nc"] = _build_program()
    nc = _CACHE["nc"]
    in_maps = _prep_inputs(**inputs)
    res = run_bass_kernel_spmd(nc, in_maps, list(range(NCORES)))
    h = np.stack([res.results[c]["hout"] for c in range(NCORES)], 0)
    return np.ascontiguousarray(
        h.reshape(B, 4 * NPC, H)).astype(np.float32)


# ------------------------------------------------------- host fallback path
def _gather_h(nodes, idx):
    return np.stack([nodes[b][idx[b]] for b in range(nodes.shape[0])], 0)


def _ln_h(x, g, b, eps=1e-6):
    mu = x.mean(-1, keepdims=True, dtype=np.float32)
    var = ((x - mu) ** 2).sum(-1, keepdims=True, dtype=np.float32) / (x.shape[-1] - 1)
    return (g * (x - mu) / (np.sqrt(var + eps) + eps) + b).astype(np.float32)


def _kernel_host(X, mask, Wv_w, Wv_b, gv, bv, We_w, We_b, ge, be,
                 Wl1, bl1, Wl2, bl2, Wl3, bl3, gl, bl):
    f32 = np.float32
    m = mask.reshape(B, N).astype(f32)
    Xc = np.ascontiguousarray(X[:, 0, :, 1, :]).astype(f32)
    m2 = np.clip(m[:, :, None] * m[:, None, :] - np.eye(N, dtype=f32), 0.0, None)
    dP = Xc[:, :, None, :] - Xc[:, None, :, :]
    D = m2 * np.sqrt((dP * dP).sum(-1, dtype=f32) + 1e-6)
    Dmask = (D + (1.0 - m2) * 10000.0).astype(f32)
    E_idx = np.argpartition(Dmask, K, axis=-1)[:, :, :K]
    dgat = np.take_along_axis(Dmask, E_idx, axis=-1)
    order = np.argsort(dgat, axis=-1, kind="stable")
    E_idx = np.take_along_axis(E_idx, order, axis=-1)
    D_nb = np.take_along_axis(dgat, order, axis=-1)
    mu_r = np.linspace(0.0, 20.0, NUM_RBF, dtype=f32)
    RBF = np.exp(-(((D_nb[..., None] - mu_r) / (20.0 / NUM_RBF)) ** 2)).astype(f32)
    ii = np.arange(N, dtype=f32)[None, :, None]
    d = (E_idx.astype(f32) - ii)[..., None] * m[:, :, None, None]
    d = np.where(np.abs(d) > SEQN, 0.0, d).astype(f32)
    freq = np.exp(np.arange(0, POS, 2, dtype=f32) * (-np.log(10000.0) / POS))
    ang = d * freq
    Ep = (np.concatenate([np.cos(ang), np.sin(ang)], -1) * (d != 0)).astype(f32)
    O = _host_geometry(Xc, m)
    mN = m[:, :, None, None]
    O_nb = _gather_h(O, E_idx) * mN
    X_nb = _gather_h(Xc, E_idx) * mN
    Om = O.reshape(B, N, 3, 3)
    Onb = O_nb.reshape(B, N, K, 3, 3)
    dXn = (X_nb - Xc[:, :, None, :]) * mN
    dU = _norm(np.einsum("bnij,bnkj->bnki", Om, dXn).astype(f32))
    Rm = np.einsum("bnji,bnkjl->bnkil", Om, Onb).astype(f32)
    Rxx, Ryy, Rzz = Rm[..., 0, 0], Rm[..., 1, 1], Rm[..., 2, 2]
    def _ss(x):
        p = x > 0
        return np.where(p, np.sqrt(np.where(p, x, 1.0)), 0.0).astype(f32)
    mags = 0.5 * _ss(np.abs(1.0 + np.stack(
        [Rxx - Ryy - Rzz, -Rxx + Ryy - Rzz, -Rxx - Ryy + Rzz], -1)))
    signs = np.sign(np.stack(
        [Rm[..., 2, 1] - Rm[..., 1, 2], Rm[..., 0, 2] - Rm[..., 2, 0],
         Rm[..., 1, 0] - Rm[..., 0, 1]], -1)).astype(f32)
    w = _ss(np.maximum(1.0 + Rxx + Ryy + Rzz, 0.0))[..., None] / 2.0
    Q = _norm(np.concatenate([signs * mags, w], -1))
    Of = (np.concatenate([dU, Q], -1) * mN).astype(f32)
    V = _host_dihedral(X.astype(f32), m)
    E = np.concatenate([Ep, RBF, Of], -1).astype(f32)
    vmask = _gather_h(m[:, :, None], E_idx)[..., 0]
    h = _ln_h(V @ Wv_w + Wv_b, gv, bv)
    h_e = _ln_h(E @ We_w + We_b, ge, be)
    for l in range(DEPTH):
        nei_v = _gather_h(h, E_idx)
        h_EV = np.concatenate(
            [np.broadcast_to(h[:, :, None, :], nei_v.shape), nei_v, h_e], -1)
        msg = np.maximum(h_EV @ Wl1[l] + bl1[l], 0.0)
        msg = np.maximum(msg @ Wl2[l] + bl2[l], 0.0)
        msg = (msg @ Wl3[l] + bl3[l]) * vmask[..., None]
        h = _ln_h(h + msg.mean(-2, dtype=np.float32), gl[l], bl[l]) * m[:, :, None]
        h = h.astype(f32)
    return h


def kernel(X, mask, Wv_w, Wv_b, gv, bv, We_w, We_b, ge, be,
           Wl1, bl1, Wl2, bl2, Wl3, bl3, gl, bl):
    args = dict(
        X=np.asarray(X, np.float32), mask=np.asarray(mask, np.float32),
        Wv_w=np.asarray(Wv_w, np.float32), Wv_b=np.asarray(Wv_b, np.float32),
        gv=np.asarray(gv, np.float32), bv=np.asarray(bv, np.float32),
        We_w=np.asarray(We_w, np.float32), We_b=np.asarray(We_b, np.float32),
        ge=np.asarray(ge, np.float32), be=np.asarray(be, np.float32),
        Wl1=np.asarray(Wl1, np.float32), bl1=np.asarray(bl1, np.float32),
        Wl2=np.asarray(Wl2, np.float32), bl2=np.asarray(bl2, np.float32),
        Wl3=np.asarray(Wl3, np.float32), bl3=np.asarray(bl3, np.float32),
        gl=np.asarray(gl, np.float32), bl=np.asarray(bl, np.float32))
    if _CACHE.get("force_host"):
        return _kernel_host(**args)
    try:
        return _kernel_device(args)
    except Exception:
        import traceback
        traceback.print_exc()
        _CACHE["force_host"] = True
        return _kernel_host(**args)
